# revision 1
# baseline (speedup 1.0000x reference)
"""Trainium2 Bass kernel for nn_MetaMultiLinear.

Math (per head h, sample b):
    w[b, k]   = sum_c cond[b, c] * CW[k, c] + cb[k]        k = o*17 + i  (544)
    out[b, o] = sum_i x1[b, i] * w[b, o*17+i]              x1 = [input, 1] (17)

Sharding: head h -> NeuronCore h (8 heads, 8 cores), full B=32768 per core.

Split i = 0..15 (needs the per-sample multiply) from i = 16 (x1 = 1, so its
contribution cond1 @ CWones^T + bias goes straight into the output
accumulator).

Per-core device algorithm (tiles of 128 samples, processed in pairs; the
group loop is a For_i hardware loop so the static program stays small —
this runtime's per-execution cost scales with static NEFF size):
  1. One DMA per group loads [cond|1|x|pad] for 2*GRP tiles.
  2. Per pair: one PE transpose (plus a 1x1 fence matmul that carries the
     semaphore waits — transpose-mode matmuls only take one sync wait)
     gives cond1^T at partitions 0-32 / 64-96; ScalarE copies PSUM->SBUF.
  3. Per tile (PE, float32r): W-MM  w1[b, o*16+i] = cond1 @ CWk^T  (K=33,
     N=512, one PSUM bank); po-MM  po[b, o] = cond1 @ CWones^T (start=True,
     opens the tile's accumulation group, carries all bias terms).
  4. Per tile (DVE, the floor: one 1x pass, 512 elem/partition): tmp =
     w1 (*) broadcast(x), reading w1 straight from PSUM.
  5. Per tile (PE, float32r): one reduce matmul with identity stationary
     streams tmp i-outer/o-inner; the PSUM out AP is a broadcast view so
     16 passes accumulate onto po[b, o] via has_written.
  6. ScalarE copies po -> SBUF (DMA cannot read PSUM); one output DMA per
     group. Reduce phases run one pair behind produce phases so the PE
     FIFO has W work while the DVE multiply runs.
"""

import sys

import numpy as np

if "/opt/trn_rl_repo" not in sys.path:
    sys.path.insert(0, "/opt/trn_rl_repo")

N_HEADS, IN_F, COND_IN, OUT_F = 8, 16, 32, 32
B = 32768
INP1 = IN_F + 1  # 17
KW = OUT_F * IN_F  # 512 (i<16 part)
C1 = COND_IN + 1  # 33
P = 128
GRP = 16  # pairs per group

_cached_nc = None

# "overlap": PE grouped reduce via overlapping PSUM out-AP (1 matmul/tile)
# "mm16":    PE grouped reduce via 16 accumulated strided matmuls (sim-safe)
REDUCE_MODE = "overlap"
# float32r: single-pass fast fp32 on PE (1 cycle/row at N>=256; exact fp32
# costs 4 cycles/row). Operands must be typed f32r at their producers.
USE_F32R = True
# use a For_i hardware loop over groups (small static program)
USE_LOOP = True


def _build_nc(b_total=B, grp=None, reps=1, loop=None):
    import concourse.bass as bass
    import concourse.mybir as mybir
    import concourse.tile as tile
    from concourse import bacc
    from contextlib import ExitStack

    f32 = mybir.dt.float32
    fr = mybir.dt.float32r if USE_F32R else f32
    if loop is None:
        loop = USE_LOOP
    nc = bacc.Bacc()
    pairs = b_total // (2 * P)
    if grp is None:
        grp = GRP
    while pairs % grp:
        grp //= 2
    groups = pairs // grp
    gsz = 2 * grp * P  # samples per group

    # cx: per sample [cond (32) | 1.0 | input (16) | zeros (15)]
    cx_t = nc.dram_tensor("cx", [b_total, 64], f32, kind="ExternalInput")
    # cwk[c, o*16+i] = CW[o*17+i, c] (i<16); row 32 = cond_bias slice
    cwk_t = nc.dram_tensor("cwk", [P, KW], fr, kind="ExternalInput")
    # cwo[c, o] = CW[o*17+16, c]; row 32 = cond_bias[o*17+16]
    cwo_t = nc.dram_tensor("cwo", [P, OUT_F], fr, kind="ExternalInput")
    ident_t = nc.dram_tensor("ident", [P, P], fr, kind="ExternalInput")
    out_t = nc.dram_tensor("out", [b_total, OUT_F], f32, kind="ExternalOutput")

    with tile.TileContext(nc) as tc, ExitStack() as ctx:
        consts = ctx.enter_context(tc.tile_pool(name="consts", bufs=1))
        ptrin = ctx.enter_context(tc.tile_pool(name="ptrin", bufs=2))
        ptrs = ctx.enter_context(tc.tile_pool(name="ptrs", bufs=4))
        ptmp = ctx.enter_context(tc.tile_pool(name="ptmp", bufs=4))
        pouts = ctx.enter_context(tc.tile_pool(name="pouts", bufs=2))
        pps_tr = ctx.enter_context(tc.tile_pool(name="pps_tr", bufs=1, space="PSUM"))
        pps_w = ctx.enter_context(tc.tile_pool(name="pps_w", bufs=3, space="PSUM"))
        pps_o = ctx.enter_context(tc.tile_pool(name="pps_o", bufs=2, space="PSUM"))

        cwk = consts.tile([P, KW], fr)
        nc.sync.dma_start(out=cwk, in_=cwk_t[:])
        cwo = consts.tile([P, OUT_F], fr)
        nc.sync.dma_start(out=cwo, in_=cwo_t[:])
        idn = consts.tile([P, P], fr)
        nc.sync.dma_start(out=idn, in_=ident_t[:])
        idn32 = idn.bitcast(f32)

        def emit_group(gb0):
            """Emit one group's program. gb0: starting sample (int or reg)."""
            trin_g = ptrin.tile([P, 2 * grp, 64], f32)
            nc.sync.dma_start(
                out=trin_g[:],
                in_=cx_t[bass.ds(gb0, gsz), :].rearrange(
                    "(t p) c -> p t c", t=2 * grp
                ),
            )
            outs_g = pouts.tile([P, 2 * grp, OUT_F], f32)

            pending = []  # (po, tmps, col)

            def emit_reduce(item):
                po, tmps, col = item
                for t in (0, 1):
                    tmp = tmps[t]
                    if REDUCE_MODE == "overlap":
                        # 16 streamed passes of 32 o-columns accumulate onto
                        # the same PSUM addresses via has_written. i-outer/
                        # o-inner keeps the dst innermost step-1/even/8B-
                        # aligned (fp32r paired PSUM write requirement).
                        rhs = tmp[:].rearrange("p o i -> p i o")
                        ov = (
                            po[:, t, 0:OUT_F]
                            .unsqueeze(1)
                            .broadcast_to([P, IN_F, OUT_F])
                        )
                        nc.tensor.matmul(
                            ov,
                            idn[:],
                            rhs,
                            start=False,
                            stop=True,
                            skip_group_check=True,
                        )
                    else:
                        tv = tmp[:].rearrange("p o i -> p i o")
                        for i in range(IN_F):
                            nc.tensor.matmul(
                                po[:, t, 0:OUT_F],
                                idn[:],
                                tv[:, i, :],
                                start=False,
                                stop=(i == IN_F - 1),
                                skip_group_check=True,
                            )
                # PSUM -> SBUF (DMA cannot read PSUM)
                nc.scalar.copy(out=outs_g[:, col : col + 2, :], in_=po[:, :, 0:OUT_F])

            for pr in range(grp):
                trin = trin_g[:, 2 * pr : 2 * pr + 2, :].rearrange("p t c -> p (t c)")
                trps = pps_tr.tile([P, P], f32)
                # Fence: carries the semaphore waits (trin DMA, idn DMA,
                # trps slot release); transpose-mode matmuls only support a
                # single sync-wait in codegen.
                nc.tensor.matmul(
                    trps[0:1, 0:1],
                    trin[:, 0:1],
                    idn32[:, 0:1],
                    start=True,
                    stop=True,
                    skip_group_check=True,
                )
                nc.tensor.transpose(trps[:], trin[:], idn32[:])
                trs = ptrs.tile([P, P], fr)
                nc.scalar.copy(out=trs[:], in_=trps[:])

                po = pps_o.tile([P, 2, 512], f32)
                tmps = []
                for t in (0, 1):
                    g = t * 64
                    cts = trs[g : g + C1, :]
                    w1 = pps_w.tile([P, KW], f32)
                    nc.tensor.matmul(
                        w1[:],
                        cts,
                        cwk[g : g + C1, :],
                        start=True,
                        stop=True,
                        tile_position=(g, 0),
                    )
                    # opens tile t's accumulation group (own PSUM bank)
                    nc.tensor.matmul(
                        po[:, t, 0:OUT_F],
                        cts,
                        cwo[g : g + C1, :],
                        start=True,
                        stop=False,
                        skip_group_check=True,
                        tile_position=(g, 0),
                    )
                    tmp = ptmp.tile([P, OUT_F, IN_F], fr)
                    w1v = w1[:].rearrange("p (o i) -> p o i", i=IN_F)
                    xv = (
                        trin[:, g + C1 : g + C1 + IN_F]
                        .unsqueeze(1)
                        .broadcast_to([P, OUT_F, IN_F])
                    )
                    nc.vector.tensor_mul(tmp[:], w1v, xv)
                    tmps.append(tmp)
                pending.append((po, tmps, 2 * pr))
                if len(pending) > 1:
                    emit_reduce(pending.pop(0))
            while pending:
                emit_reduce(pending.pop(0))
            nc.sync.dma_start(
                out=out_t[bass.ds(gb0, gsz), :].rearrange(
                    "(t p) o -> p t o", t=2 * grp
                ),
                in_=outs_g[:],
            )

        if loop and groups > 1:
            if reps == 1:
                with tc.For_i(0, groups * gsz, gsz) as iv:
                    emit_group(iv)
            else:
                with tc.For_i(0, reps, 1):
                    with tc.For_i(0, groups * gsz, gsz) as iv:
                        emit_group(iv)
        else:
            for it in range(groups * reps):
                emit_group((it % groups) * gsz)

    nc.compile()
    return nc


def _get_nc():
    global _cached_nc
    if _cached_nc is None:
        _cached_nc = _build_nc()
    return _cached_nc


def _make_in_maps(input, cond, cond_weight, cond_bias):
    ident = np.eye(P, dtype=np.float32)
    in_maps = []
    n_heads, b_total = input.shape[0], input.shape[1]
    for h in range(n_heads):
        cx = np.zeros((b_total, 64), np.float32)
        cx[:, :COND_IN] = cond[h]
        cx[:, COND_IN] = 1.0
        cx[:, C1 : C1 + IN_F] = input[h]
        cw3 = cond_weight[h].reshape(OUT_F, INP1, COND_IN)  # (o, i, c)
        cb2 = cond_bias[h].reshape(OUT_F, INP1)  # (o, i)
        cwk = np.zeros((P, KW), np.float32)
        cwk1 = cw3[:, :IN_F, :].transpose(2, 0, 1).reshape(COND_IN, KW)
        cwk[0:COND_IN] = cwk1
        cwk[COND_IN] = cb2[:, :IN_F].reshape(KW)
        cwk[64 : 64 + COND_IN] = cwk1
        cwk[64 + COND_IN] = cb2[:, :IN_F].reshape(KW)
        cwo = np.zeros((P, OUT_F), np.float32)
        cwo[0:COND_IN] = cw3[:, IN_F, :].T  # [c, o]
        cwo[COND_IN] = cb2[:, IN_F]
        cwo[64 : 64 + COND_IN] = cw3[:, IN_F, :].T
        cwo[64 + COND_IN] = cb2[:, IN_F]
        in_maps.append({"cx": cx, "cwk": cwk, "cwo": cwo, "ident": ident})
    return in_maps


def _run(in_maps, **kwargs):
    from concourse import bass_utils

    nc = _get_nc()
    return bass_utils.run_bass_kernel_spmd(
        nc, in_maps, core_ids=list(range(N_HEADS)), **kwargs
    )


def kernel(input, cond, cond_weight, cond_bias):
    input = np.asarray(input, np.float32)
    cond = np.asarray(cond, np.float32)
    cond_weight = np.asarray(cond_weight, np.float32)
    cond_bias = np.asarray(cond_bias, np.float32)
    in_maps = _make_in_maps(input, cond, cond_weight, cond_bias)
    res = _run(in_maps)
    return np.stack([r["out"] for r in res.results], axis=0)



# revision 3
# speedup vs baseline: 2.2764x; 2.2764x over previous
"""Trainium2 Bass kernel for nn_MetaMultiLinear (bf16, transpose-free, v3).

Math (per head h, sample b):
    w[b, k]   = sum_c cond[b, c] * CW[k, c] + cb[k]        k = o*17 + i  (544)
    out[b, o] = sum_i x1[b, i] * w[b, o*17+i]              x1 = [input, 1] (17)

Sharding: head h -> NeuronCore h (8 heads, 8 cores), full B=32768 per core.

All data bf16 (fp32 PSUM accumulation; harness gate 2e-2, bf16 lands ~5e-3).
The PE here runs at a fixed 1.2 GHz (every matmul measures the cold-state
formula (219+N)/1.2 regardless of activity), so the design minimizes PE
streamed columns and splits the i-reduction between PE and DVE.

Host pre-packs everything in on-chip layout (partition-contiguous DMAs),
including cond1^T, so there is no on-chip transpose:
  ct  [128, NP, 128]: pair p, col j: rows 0:33 = [cond|1]^T of sample
      256p+j (tile 2p), rows 64:97 = same for tile 2p+1.
  xs  [128, NP, 32]:  xs[p, pr, t*16+i] = input[256pr + 128t + p, i]
  cwk [128, 512]:     rows g:g+33 (g=0,64) = CW[(i<16, o), c] c-major,
      i-MAJOR columns (k' = i*32+o), bias row at g+32.
  cwo [128, 32]:      i=16 (x1=1) slice; opens the out accumulation group.
  out [128, NP, 64]   bf16 (t*32+o), host converts back to fp32 [B, 32].

Fully unrolled (no For_i: its back-edge barrier blocked DMA prefetch and
idled the PE ~7 us/iteration). DMAs issued per 8-pair chunk to spread
across the 16 DMA queues.

Per pair (2 tiles of 128 samples):
  1. W-MM per tile (PE, bf16, N=512, K=33 at row strips 0/64: the two
     tiles' matmuls run concurrently); po-MM per tile (N=32, also strip-
     concurrent) computes the i=16 + bias contribution.
  2. DVE pair-fused mul tmp[b,(t,i,o)] = w1 (*) bcast(x) straight from
     PSUM fp32 (1x mode; layout (i,o) so downstream is contiguous).
  3. Reduce over i, route per pair:
     P: PE identity-matmul streams tmp (N=512 contiguous) accumulating
        onto po[b,o] via the broadcast out-AP; ACT copies po -> outs.
     D: DVE binary-tree adds (2x mode, contiguous 32-wide o rows), ACT
        copies po -> SBUF, DVE final add writes outs.
"""

import sys

import numpy as np

if "/opt/trn_rl_repo" not in sys.path:
    sys.path.insert(0, "/opt/trn_rl_repo")

N_HEADS, IN_F, COND_IN, OUT_F = 8, 16, 32, 32
B = 32768
INP1 = IN_F + 1  # 17
KW = OUT_F * IN_F  # 512 (i<16 part)
C1 = COND_IN + 1  # 33
P = 128
NT = B // P  # 256 tiles
NP = NT // 2  # 128 pairs
DMA_CHUNK = 8  # pairs per DMA

# reduce route per pair, cycled: P = PE identity-matmul, D = DVE tree
PAIR_ROUTES = "PDPDD"

_cached_nc = None


def _build_nc(n_pairs=NP):
    import concourse.bass as bass
    import concourse.mybir as mybir
    import concourse.tile as tile
    from concourse import bacc
    from contextlib import ExitStack

    f32 = mybir.dt.float32
    bf16 = mybir.dt.bfloat16
    nc = bacc.Bacc()

    ct_t = nc.dram_tensor("ct", [P, n_pairs, P], bf16, kind="ExternalInput")
    xs_t = nc.dram_tensor("xs", [P, n_pairs, 2 * IN_F], bf16, kind="ExternalInput")
    cwk_t = nc.dram_tensor("cwk", [P, KW], bf16, kind="ExternalInput")
    cwo_t = nc.dram_tensor("cwo", [P, OUT_F], bf16, kind="ExternalInput")
    ident_t = nc.dram_tensor("ident", [P, P], bf16, kind="ExternalInput")
    out_t = nc.dram_tensor("out", [P, n_pairs, 2 * OUT_F], bf16, kind="ExternalOutput")

    n_chunks = n_pairs // DMA_CHUNK

    with tile.TileContext(nc) as tc, ExitStack() as ctx:
        consts = ctx.enter_context(tc.tile_pool(name="consts", bufs=1))
        pin = ctx.enter_context(tc.tile_pool(name="pin", bufs=3))
        ptmp = ctx.enter_context(tc.tile_pool(name="ptmp", bufs=3))
        ptree = ctx.enter_context(tc.tile_pool(name="ptree", bufs=2))
        pposb = ctx.enter_context(tc.tile_pool(name="pposb", bufs=2))
        pouts = ctx.enter_context(tc.tile_pool(name="pouts", bufs=3))
        pps_w = ctx.enter_context(tc.tile_pool(name="pps_w", bufs=2, space="PSUM"))
        pps_o = ctx.enter_context(tc.tile_pool(name="pps_o", bufs=2, space="PSUM"))

        cwk = consts.tile([P, KW], bf16)
        nc.sync.dma_start(out=cwk, in_=cwk_t[:])
        cwo = consts.tile([P, OUT_F], bf16)
        nc.sync.dma_start(out=cwo, in_=cwo_t[:])
        idn = consts.tile([P, P], bf16)
        nc.sync.dma_start(out=idn, in_=ident_t[:])

        for ch in range(n_chunks):
            p0 = ch * DMA_CHUNK
            ct_c = pin.tile([P, DMA_CHUNK, P], bf16, tag="ct_c")
            nc.sync.dma_start(out=ct_c[:], in_=ct_t[:, p0 : p0 + DMA_CHUNK, :])
            xs_c = pin.tile([P, DMA_CHUNK, 2 * IN_F], bf16, tag="xs_c")
            nc.sync.dma_start(out=xs_c[:], in_=xs_t[:, p0 : p0 + DMA_CHUNK, :])
            outs_c = pouts.tile([P, DMA_CHUNK, 2 * OUT_F], bf16)

            for j in range(DMA_CHUNK):
                pr = p0 + j
                route = PAIR_ROUTES[pr % len(PAIR_ROUTES)]
                cts = ct_c[:, j, :]
                xst = xs_c[:, j, :].rearrange("p (t i) -> p t i", t=2)
                w1 = pps_w.tile([P, 2, KW], f32)
                po = pps_o.tile([P, 2, 512], f32)
                # hypernet: per-tile W and po matmuls, strip-concurrent
                for t in (0, 1):
                    g = 64 * t
                    ctsl = cts[g : g + C1, :]
                    nc.tensor.matmul(
                        w1[:, t, :],
                        ctsl,
                        cwk[g : g + C1, :],
                        start=True,
                        stop=True,
                        tile_position=(g, 0),
                    )
                    nc.tensor.matmul(
                        po[:, t, 0:OUT_F],
                        ctsl,
                        cwo[g : g + C1, :],
                        start=True,
                        stop=(route == "D"),
                        skip_group_check=True,
                        tile_position=(g, 0),
                    )
                # pair-fused multiply from PSUM, (i,o) layout
                tmp = ptmp.tile([P, 2, IN_F, OUT_F], bf16)
                w1v = w1[:].rearrange("p t (i o) -> p t i o", o=OUT_F)
                xv = (
                    xst[:]
                    .unsqueeze(3)
                    .broadcast_to([P, 2, IN_F, OUT_F])
                )
                nc.vector.tensor_mul(tmp[:], w1v, xv)
                ov = outs_c[:, j, :].rearrange("p (t o) -> p t o", t=2)
                if route == "P":
                    for t in (0, 1):
                        pv = (
                            po[:, t, 0:OUT_F]
                            .unsqueeze(1)
                            .broadcast_to([P, IN_F, OUT_F])
                        )
                        nc.tensor.matmul(
                            pv,
                            idn[:],
                            tmp[:, t, :, :],
                            start=False,
                            stop=True,
                            skip_group_check=True,
                        )
                    nc.scalar.copy(out=ov, in_=po[:, :, 0:OUT_F])
                else:
                    r8 = ptree.tile([P, 2, 8, OUT_F], bf16, tag="r8")
                    nc.vector.tensor_add(r8[:], tmp[:, :, 0:8, :], tmp[:, :, 8:16, :])
                    r4 = ptree.tile([P, 2, 4, OUT_F], bf16, tag="r4")
                    nc.vector.tensor_add(r4[:], r8[:, :, 0:4, :], r8[:, :, 4:8, :])
                    r2 = ptree.tile([P, 2, 2, OUT_F], bf16, tag="r2")
                    nc.vector.tensor_add(r2[:], r4[:, :, 0:2, :], r4[:, :, 2:4, :])
                    r1 = ptree.tile([P, 2, 1, OUT_F], bf16, tag="r1")
                    nc.vector.tensor_add(r1[:], r2[:, :, 0:1, :], r2[:, :, 1:2, :])
                    poS = pposb.tile([P, 2, OUT_F], bf16)
                    nc.scalar.copy(out=poS[:], in_=po[:, :, 0:OUT_F])
                    nc.vector.tensor_add(ov, r1[:, :, 0, :], poS[:])

            nc.sync.dma_start(
                out=out_t[:, p0 : p0 + DMA_CHUNK, :],
                in_=outs_c[:],
            )

    nc.compile()
    return nc


def _get_nc():
    global _cached_nc
    if _cached_nc is None:
        _cached_nc = _build_nc()
    return _cached_nc


def _make_in_maps(input, cond, cond_weight, cond_bias):
    import ml_dtypes

    bf = ml_dtypes.bfloat16
    ident = np.eye(P, dtype=bf)
    in_maps = []
    n_heads, b_total = input.shape[0], input.shape[1]
    npair = b_total // 256
    for h in range(n_heads):
        cond1 = np.concatenate(
            [cond[h], np.ones((b_total, 1), np.float32)], axis=1
        )  # [B, 33]
        c4 = cond1.reshape(npair, 2, P, C1).astype(bf)  # [p, t, j, c]
        ct = np.zeros((P, npair, P), bf)
        ct[0:C1] = c4[:, 0].transpose(2, 0, 1)
        ct[64 : 64 + C1] = c4[:, 1].transpose(2, 0, 1)
        xs = (
            input[h]
            .reshape(npair, 2, P, IN_F)
            .transpose(2, 0, 1, 3)
            .reshape(P, npair, 2 * IN_F)
            .astype(bf)
        )
        cw3 = cond_weight[h].reshape(OUT_F, INP1, COND_IN)  # (o, i, c)
        cb2 = cond_bias[h].reshape(OUT_F, INP1)  # (o, i)
        # i-major columns: cwk[c, i*32+o]
        cwk = np.zeros((P, KW), bf)
        cwk1 = cw3[:, :IN_F, :].transpose(2, 1, 0).reshape(COND_IN, KW).astype(bf)
        cwk[0:COND_IN] = cwk1
        cwk[COND_IN] = cb2[:, :IN_F].T.reshape(KW).astype(bf)
        cwk[64 : 64 + COND_IN] = cwk1
        cwk[64 + COND_IN] = cb2[:, :IN_F].T.reshape(KW).astype(bf)
        cwo = np.zeros((P, OUT_F), bf)
        cwo[0:COND_IN] = cw3[:, IN_F, :].T.astype(bf)
        cwo[COND_IN] = cb2[:, IN_F].astype(bf)
        cwo[64 : 64 + COND_IN] = cw3[:, IN_F, :].T.astype(bf)
        cwo[64 + COND_IN] = cb2[:, IN_F].astype(bf)
        in_maps.append({"ct": ct, "xs": xs, "cwk": cwk, "cwo": cwo, "ident": ident})
    return in_maps


def _run(in_maps, **kwargs):
    from concourse import bass_utils

    nc = _get_nc()
    return bass_utils.run_bass_kernel_spmd(
        nc, in_maps, core_ids=list(range(N_HEADS)), **kwargs
    )


def _unpack_out(res):
    # out [128, NP, 64] bf16 -> [B, 32] fp32 per head
    outs = []
    for r in res.results:
        o = np.asarray(r["out"], dtype=np.float32)  # [P, NP, 64]
        o = o.reshape(P, -1, 2, OUT_F).transpose(1, 2, 0, 3).reshape(-1, OUT_F)
        outs.append(o)
    return np.stack(outs, axis=0)


def kernel(input, cond, cond_weight, cond_bias):
    input = np.asarray(input, np.float32)
    cond = np.asarray(cond, np.float32)
    cond_weight = np.asarray(cond_weight, np.float32)
    cond_bias = np.asarray(cond_bias, np.float32)
    in_maps = _make_in_maps(input, cond, cond_weight, cond_bias)
    res = _run(in_maps)
    return _unpack_out(res)
